# revision 20
# baseline (speedup 1.0000x reference)
"""Trainium2 Bass kernel for the Capsule routing layer (nn_Capsule_49658411876931).

Math (see reference):
    u_hat[b,j,i,d] = sum_k W[j,i,d,k] * x[b,i,k]
    b0 = 0
    for r in 0..2:
        c = softmax(b, axis=j)
        s[b,j,d] = sum_i c[b,j,i] u_hat[b,j,i,d]
        v = squash(s)  (over d)
        if r < 2: b += sum_d u_hat[b,j,i,d] v[b,j,d]
    return v  [B, J, D]

Sharding: batch B=32 split over 8 cores (B_LOC=4); W replicated (16.8 MB
bf16 per core, streamed once from HBM in 1 MB chunks at ~340 GB/s). The
routing loop is then fully core-local: no collectives at all (the previous
I-sharded design lost ~90 us to three ncfw AllReduce latencies).

Per-core layouts (P = SBUF partition index):
  i = ((g*2 + rp)*4 + c)*8 + i8   (g in 0..31, rp in 0..1, c in 0..3, i8 in 0..7)
  u_hat "C" tensor : [P = 32c + 4*i8 + b, free = (g, rp, d, j)]  bf16
  logits / c       : [P = 32c + 4*i8 + b, free = (g, rp, j)]
u_hat is computed with 8-way tile_position-packed PE matmuls: stationary is a
host-prepped block-diagonal x chunk [64 rows = (i8,k), 32 cols = (i8',b)]
(zero off-diagonal), moving is the W chunk [64, 512]; each matmul covers 8
input capsules, 8 matmuls (rp, c) run concurrently per round g.

s[b,dj] = sum_i c*u_hat runs on the PE as 64 accumulating matmuls against a
0/1 "collapse the 32 (c,i8)-strips" selector stationary (delta_{p%4,b}); for
iteration 0 the softmax coefficients are the constant 1/J, folded into a
(1/J)-scaled selector so no elementwise pass is needed. The agreement
d-contraction runs as a log2 tree of bf16 2x-mode tensor_tensor adds.
squash's sqrt uses exp(0.5*log(.)) so the ACT engine stays on the
natural_log_exp table set the whole kernel (no 2.7us table reloads).
"""

import numpy as np
import ml_dtypes

import concourse.bass as bass
import concourse.tile as tile
from concourse import bacc, mybir
from concourse.bass_utils import run_bass_kernel_spmd

F32 = mybir.dt.float32
BF16 = mybir.dt.bfloat16
FP8 = mybir.dt.float8e4
Alu = mybir.AluOpType
Act = mybir.ActivationFunctionType

B, I, K = 32, 2048, 8
J, D = 32, 16
JD = J * D                     # 512
NCORES = 8
B_LOC = B // NCORES            # 4
NG = 32                        # rounds g; i = ((g*2+rp)*4+c)*8 + i8
CHUNK_G = 4                    # g's per W DMA chunk (2 MB bf16 / 1 MB fp8)
NCHUNK = NG // CHUNK_G         # 8
W_FP8 = False                  # fp8 W fails the 2e-2 gate (measured 2.5e-2)
W_SCALE = 64.0                 # power-of-2, folded exactly into bf16 xs
GB = 8                         # g's per routing block
NBLK = NG // GB                # 4 routing blocks (16 (g,rp)-slices each)
ROUTINGS = 3
EPS = 1e-7

_CACHE = {}


def _build():
    nc = bacc.Bacc("TRN2", target_bir_lowering=False, debug=False, num_devices=NCORES)

    wt_in = nc.dram_tensor(
        "wt", [NCHUNK, 128, CHUNK_G, 4, JD], FP8 if W_FP8 else BF16,
        kind="ExternalInput",
    )
    xs_in = nc.dram_tensor("xs", [128, NG, 4, 32], BF16, kind="ExternalInput")
    v_out = nc.dram_tensor("v", [B_LOC, J, D], F32, kind="ExternalOutput")

    # Selector constants for cross-partition PE ops:
    #   sel[p, b'] = 1 iff p % 4 == b'     (collapse the 32 (c,i8)-strips)
    #   sel32 = sel / J                    (fold in the uniform iter-0 softmax)
    #   selT[b, p] = sel^T                 (replicate over the 32 strips)
    p_idx = np.arange(128)
    sel_np = (p_idx[:, None] % B_LOC == np.arange(B_LOC)[None, :]).astype(np.float32)
    selpack = np.zeros((128, 2 * B_LOC + 128), np.float32)
    selpack[:, 0:B_LOC] = sel_np
    selpack[:, B_LOC : 2 * B_LOC] = sel_np / J
    selpack[0:B_LOC, 2 * B_LOC :] = sel_np.T
    sel_dram = nc.inline_tensor(selpack.astype(ml_dtypes.bfloat16), "selpack")

    with tile.TileContext(nc) as tc:
        with (
            tc.tile_pool(name="persist", bufs=1) as pp,
            tc.tile_pool(name="small", bufs=1) as sp,
            tc.tile_pool(name="spsum", bufs=1, space="PSUM") as ssp,
        ):
            # ---- persistent SBUF tensors ----
            xs = pp.tile([128, NG, 4, 32], BF16)        # block-diag x stationaries
            C = pp.tile([128, NG, 2, D, J], BF16)       # u_hat
            bl = pp.tile([128, NG, 2, J], F32)          # routing logits
            p_t = pp.tile([128, NG, 2, J], BF16)        # exp(b)
            selc = pp.tile([128, 2 * B_LOC + 128], BF16)
            v_rep = pp.tile([128, D, J], BF16)          # v replicated over strips

            sel = selc[:, 0:B_LOC]
            sel32 = selc[:, B_LOC : 2 * B_LOC]
            selT = selc[0:B_LOC, 2 * B_LOC :]

            # persistent PSUM: s accumulator + v/fac replication banks
            s_ps = ssp.tile([B_LOC, D * J], F32)
            vr_ps = ssp.tile([128, D * J], F32)
            fr_ps = ssp.tile([128, J], F32)

            # xs halves go first (phase-1 matmuls gate on them); W chunks chase
            nc.gpsimd.dma_start(xs[:, 0 : NG // 2], xs_in[:, 0 : NG // 2])
            nc.gpsimd.dma_start(xs[:, NG // 2 :], xs_in[:, NG // 2 :])
            nc.sync.dma_start(selc[:], sel_dram[:])
            nc.vector.memset(bl[:], 0.0)

            # Warm the ACT natural_log_exp table set at t~0 (under the W DMA
            # shadow) so the first squash/softmax doesn't stall ~2.7us.
            wa = sp.tile([1, 8], F32, tag="wa")
            wb = sp.tile([1, 8], F32, tag="wb")
            nc.vector.memset(wa[:], 1.0)
            nc.scalar.activation(wb[:], wa[:], Act.Ln)
            nc.scalar.activation(wa[:], wb[:], Act.Exp)

            # ---- phase 1: u_hat + iteration-0 s accumulation ----
            ns0 = [0]

            def s0_slice(g, rp):
                kk = ns0[0]
                ns0[0] += 1
                nc.tensor.matmul(
                    s_ps[:],
                    sel32,
                    C[:, g, rp].rearrange("p d j -> p (d j)"),
                    start=(kk == 0),
                    stop=(kk == 2 * NG - 1),
                )

            with (
                tc.tile_pool(name="wpool", bufs=3) as wp,
                tc.tile_pool(name="psum1", bufs=2, space="PSUM") as ps1,
            ):
                def w_dma(ch):
                    wt = wp.tile([128, CHUNK_G, 4, JD], FP8 if W_FP8 else BF16, tag="wt")
                    # all W DMA issues on sync: the scalar queue is busy with
                    # the PSUM->SBUF casts and would delay chunk issue
                    nc.sync.dma_start(wt[:], wt_in[ch])
                    return wt

                wts = [w_dma(0), w_dma(1)]
                nsub = 0
                for ch in range(NCHUNK):
                    wt = wts.pop(0)
                    if ch + 2 < NCHUNK:
                        wts.append(w_dma(ch + 2))
                    for gg in range(CHUNK_G):
                        g = ch * CHUNK_G + gg
                        pg = ps1.tile([128, 2, JD], F32, tag="pg")
                        for rp in range(2):
                            for c in range(4):
                                nc.tensor.matmul(
                                    pg[32 * c : 32 * c + 32, rp, :],
                                    xs[64 * rp : 64 * rp + 64, g, c, :],
                                    wt[64 * rp : 64 * rp + 64, gg, c, :],
                                    tile_position=(64 * rp, 32 * c),
                                )
                        dst = C[:, g].rearrange("p rp d j -> p rp (d j)")
                        if g % 2 == 0:
                            nc.scalar.copy(dst, pg[:])
                        else:
                            nc.vector.tensor_copy(dst, pg[:])
                        nsub += 1
                        if nsub > 2:
                            g2 = nsub - 3
                            s0_slice(g2, 0)
                            s0_slice(g2, 1)
                for g2 in range(NG - 2, NG):
                    s0_slice(g2, 0)
                    s0_slice(g2, 1)

            # ---- routing ----
            with tc.tile_pool(name="blk", bufs=2) as bp:
                for it in range(ROUTINGS):
                    # squash factor from s_ps (all on [B_LOC(=4), ...])
                    s_gb = sp.tile([B_LOC, D, J], BF16, tag="s_gb")
                    nc.scalar.copy(s_gb.rearrange("b d j -> b (d j)"), s_ps[:])
                    sq = sp.tile([B_LOC, D, J], F32, tag="sq")
                    nc.scalar.activation(
                        sq.rearrange("b d j -> b (d j)"), s_ps[:], Act.Square
                    )
                    n2 = sp.tile([B_LOC, J], F32, tag="n2")
                    nc.vector.tensor_reduce(
                        n2[:],
                        sq.rearrange("b d j -> b j d"),
                        axis=mybir.AxisListType.X,
                        op=Alu.add,
                    )
                    # factor = n2 / (1 + n2) / sqrt(n2 + eps); sqrt via
                    # exp(0.5 * ln(.)) to stay on the exp/ln table set
                    n2e = sp.tile([B_LOC, J], F32, tag="n2e")
                    nc.vector.tensor_scalar_add(n2e[:], n2[:], EPS)
                    lg = sp.tile([B_LOC, J], F32, tag="lg")
                    nc.scalar.activation(lg[:], n2e[:], Act.Ln)
                    sd = sp.tile([B_LOC, J], F32, tag="sd")
                    nc.scalar.activation(sd[:], lg[:], Act.Exp, scale=0.5)
                    tmp = sp.tile([B_LOC, J], F32, tag="tmp")
                    nc.vector.tensor_scalar_add(tmp[:], n2[:], 1.0)
                    nc.vector.tensor_tensor(tmp[:], tmp[:], sd[:], op=Alu.mult)
                    fac = sp.tile([B_LOC, J], F32, tag="fac")
                    nc.vector.reciprocal(fac[:], tmp[:])

                    if it < ROUTINGS - 1:
                        facb = sp.tile([B_LOC, J], BF16, tag="facb")
                        nc.vector.tensor_tensor(facb[:], fac[:], n2[:], op=Alu.mult)
                        # replicate s and fac over the 32 strips via the PE
                        nc.tensor.matmul(
                            vr_ps[:], selT, s_gb.rearrange("b d j -> b (d j)")
                        )
                        nc.tensor.matmul(fr_ps[:], selT, facb[:])
                        fr_sb = sp.tile([128, J], BF16, tag="fr_sb")
                        nc.scalar.copy(fr_sb[:], fr_ps[:])
                        nc.vector.tensor_tensor(
                            v_rep[:],
                            vr_ps.rearrange("p (d j) -> p d j", d=D, j=J),
                            fr_sb[:, None, :].broadcast_to([128, D, J]),
                            op=Alu.mult,
                        )
                        # pass A per block: agreement (log2 tree over d),
                        # logit update, exp
                        for blk in range(NBLK):
                            gs = slice(blk * GB, blk * GB + GB)
                            Cb = C[:, gs]
                            pi2 = bp.tile([128, GB, 2, D, J], BF16, tag="pi2")
                            nc.vector.tensor_tensor(
                                pi2[:],
                                Cb,
                                v_rep[:, None, None, :, :].broadcast_to(
                                    [128, GB, 2, D, J]
                                ),
                                op=Alu.mult,
                            )
                            t8 = bp.tile([128, GB, 2, 8, J], BF16, tag="t8")
                            nc.vector.tensor_tensor(
                                t8[:], pi2[:, :, :, 0:8, :], pi2[:, :, :, 8:16, :],
                                op=Alu.add,
                            )
                            t4 = bp.tile([128, GB, 2, 4, J], BF16, tag="t4")
                            nc.vector.tensor_tensor(
                                t4[:], t8[:, :, :, 0:4, :], t8[:, :, :, 4:8, :],
                                op=Alu.add,
                            )
                            t2 = bp.tile([128, GB, 2, 2, J], BF16, tag="t2")
                            nc.vector.tensor_tensor(
                                t2[:], t4[:, :, :, 0:2, :], t4[:, :, :, 2:4, :],
                                op=Alu.add,
                            )
                            t1 = bp.tile([128, GB, 2, J], F32, tag="t1")
                            nc.vector.tensor_tensor(
                                t1[:], t2[:, :, :, 0, :], t2[:, :, :, 1, :],
                                op=Alu.add,
                            )
                            nc.gpsimd.tensor_tensor(
                                bl[:, gs], bl[:, gs], t1[:], op=Alu.add
                            )
                            nc.scalar.activation(p_t[:, gs], bl[:, gs], Act.Exp)
                        # softmax finish: 1/sum_j exp is folded into the
                        # s-matmul selector (per-partition-and-slice scale),
                        # so no full c = p_t/S elementwise pass is needed.
                        S = sp.tile([128, NG, 2], F32, tag="S")
                        nc.vector.tensor_reduce(
                            S[:], p_t[:], axis=mybir.AxisListType.X, op=Alu.add
                        )
                        Sr = sp.tile([128, NG, 2], BF16, tag="Sr")
                        with nc.allow_low_precision(
                            reason="softmax divisor in bf16: c is consumed in "
                            "bf16 products anyway"
                        ):
                            nc.vector.reciprocal(Sr[:], S[:])
                        sel_s = sp.tile([128, NG, 2, B_LOC], BF16, tag="sel_s")
                        nc.vector.tensor_tensor(
                            sel_s[:],
                            sel[:, None, None, :].broadcast_to([128, NG, 2, B_LOC]),
                            Sr[:, :, :, None].broadcast_to([128, NG, 2, B_LOC]),
                            op=Alu.mult,
                        )
                        # pass B per block: p_t product + s matmuls (the last
                        # block's product is split so its s-matmuls retire
                        # right behind the DVE instead of 16-at-the-end)
                        for blk in range(NBLK):
                            g0 = blk * GB
                            nsplit = 4 if blk == NBLK - 1 else 1
                            for sb in range(nsplit):
                                gw = GB // nsplit
                                gs = slice(g0 + sb * gw, g0 + (sb + 1) * gw)
                                pi = bp.tile([128, GB, 2, D, J], BF16, tag="pi")
                                piv = pi[:, 0 : gw]
                                nc.vector.tensor_tensor(
                                    piv,
                                    C[:, gs],
                                    p_t[:, gs, :, None, :].broadcast_to(
                                        [128, gw, 2, D, J]
                                    ),
                                    op=Alu.mult,
                                )
                                for gr in range(gw * 2):
                                    g2, rp = divmod(gr, 2)
                                    kk = (g0 + sb * gw) * 2 + gr
                                    nc.tensor.matmul(
                                        s_ps[:],
                                        sel_s[:, g0 + sb * gw + g2, rp, :],
                                        piv[:, g2, rp].rearrange(
                                            "p d j -> p (d j)"
                                        ),
                                        start=(kk == 0),
                                        stop=(kk == 2 * NG - 1),
                                    )
                    else:
                        # final output: v = s * fac in f32, (d, j) -> (j, d)
                        facf = sp.tile([B_LOC, J], F32, tag="facf")
                        nc.vector.tensor_tensor(
                            facf[:], fac[:], n2[:], op=Alu.mult
                        )
                        v_jd = sp.tile([B_LOC, J, D], F32, tag="v_jd")
                        nc.vector.tensor_tensor(
                            v_jd[:],
                            s_gb.rearrange("b d j -> b j d"),
                            facf[:, :, None].broadcast_to([B_LOC, J, D]),
                            op=Alu.mult,
                        )
                        nc.sync.dma_start(v_out[:], v_jd[:])

    nc.compile()
    return nc


def _prep_inputs(x, W):
    """Host-side layout prep (bf16). W is shared by all cores; x is B-sliced."""
    # wt[ch, p=(rp,i8,k), gg, c, (d j)] = W[j, i, d, k],
    # i = ((g*2+rp)*4+c)*8 + i8, g = ch*CHUNK_G + gg
    Wr = W.reshape(J, NG, 2, 4, 8, D, K)                  # j g rp c i8 d k
    wt = np.ascontiguousarray(Wr.transpose(1, 2, 4, 6, 3, 5, 0))  # g rp i8 k c d j
    wt = wt.reshape(NCHUNK, CHUNK_G, 128, 4, JD).transpose(0, 2, 1, 3, 4)
    if W_FP8:
        wt = np.ascontiguousarray(wt * W_SCALE).astype(ml_dtypes.float8_e4m3)
    else:
        wt = np.ascontiguousarray(wt).astype(ml_dtypes.bfloat16)
    i8 = np.arange(8)
    if W_FP8:
        x = x / W_SCALE  # exact in bf16 (power-of-2); undoes the W scale
    in_maps = []
    for m in range(NCORES):
        xb = x[B_LOC * m : B_LOC * (m + 1)].reshape(B_LOC, NG, 2, 4, 8, K)
        xsz = np.zeros((2, 8, K, NG, 4, 8, B_LOC), np.float32)  # rp i8 k g c i8' b
        xsz[:, i8, :, :, :, i8, :] = xb.transpose(4, 2, 5, 1, 3, 0)[i8]
        in_maps.append(
            {
                "wt": wt,
                "xs": xsz.reshape(128, NG, 4, 32).astype(ml_dtypes.bfloat16),
            }
        )
    return in_maps


def run(inputs, trace=False):
    if "nc" not in _CACHE:
        _CACHE["nc"] = _build()
    nc = _CACHE["nc"]
    in_maps = _prep_inputs(np.asarray(inputs["x"]), np.asarray(inputs["W"]))
    bkr = run_bass_kernel_spmd(
        nc, in_maps, core_ids=list(range(NCORES)), trace=trace
    )
    out = np.concatenate(
        [bkr.results[m]["v"].astype(np.float32) for m in range(NCORES)], axis=0
    )
    return out, bkr


def kernel(x, W):
    out, _ = run({"x": np.asarray(x), "W": np.asarray(W)})
    return out
